# revision 20
# baseline (speedup 1.0000x reference)
"""Trainium2 Bass kernel for nn_Attention (GQA with group-summed query heads).

Algorithm notes (validated against reference in numpy):
- The reference einsum 'bghnd,bhsd->bhns' SUMS over the query-group axis, so the
  16 query heads collapse into 4 effective heads: wq columns can be pre-summed
  per kv-head (RoPE is linear per-position, both /sqrt(64) scalings folded in).
- This makes the problem plain 4-head attention: 2 batches x 4 kv-heads = 8
  independent (b,h) attention instances -> one per NeuronCore.
- Head dims are pair-permuted [t1(even), t2(odd)] so RoPE becomes wide
  elementwise multiply-adds: P1 = W1.T@xT (q/k stacked on 32-row blocks),
  P2 = signed pair-swap of P1 done by ONE permutation-matrix matmul, then
  rot = P1*[c;c;c;c] + P2*[s;s;s;s].
- Scores are computed TRANSPOSED (scoresT[key, query]) so exp(scoresT) is
  directly usable as the AV matmul's rhs with V as lhsT; an all-ones column
  appended to V yields the softmax denominators for free.
- No max-subtraction needed: scores = q_eff . k / 64 with |scores| <~ 1.5.

v2 performance changes vs v1:
- Input DMAs batched into ~2MB transfers spread across THREE queues
  (sync HWDGE / scalar HWDGE / gpsimd SWDGE): the per-DMA completion-receipt
  serialization on one FIFO queue made v1's loads ~74us; now ~25us.
- exp() is split between the Scalar engine (true exp activation) and the
  Vector engine using a one-instruction bf16 Schraudolph approximation:
  bits16 = int16(s*(2^7*log2e) + (127*2^7 - c)) bitcast to bf16 ~= exp(s).
  HW int16 convert is round-to-nearest (probed); the sawtooth error (~3% max)
  mostly cancels through the softmax normalization (rel err ~1.2e-2 total).
  ACT:DVE group ratio ~3:2 balances both engines' busy time.
- V projection runs column-tiled: two s-blocks' V matmuls execute
  CONCURRENTLY in PE col-groups (0,0)/(0,64), halving V-proj PE time.
- Output is written bf16 via gpsimd casting DMAs (host upcasts to f32).
"""

import sys
import os

sys.path.insert(0, "/opt/trn_rl_repo")

import numpy as np
import ml_dtypes

B, S, D = 2, 4096, 1024
QH, KVH, HD = 16, 4, 64
KV_DIM = KVH * HD  # 256
NCORES = 8
SB = 512           # s-block / q-block width
NSB = S // SB      # 8
GRP = 3            # key-chunks (128 keys each) per exp group -> [128, 1536]

# Schraudolph bf16 constants (HW int16 convert = round-to-nearest, probed)
_LOG2E = 1.4426950408889634
SCH_C1 = float(np.float32(_LOG2E * 128.0))
SCH_C2 = float(np.float32(127.0 * 128.0 - 0.057985 * 128.0))
# exp engine schedule per group index: True = ACT exp, False = DVE schraudolph
EXP_PATTERN = [True, True, False]  # 2:1 ACT:DVE (DVE psum reads are ~2x slower)

_CACHE = {}


def _build_nc(stop_after="D", collective=True, reps=1, exp_pattern=None, pipelined=True, out_f32=False):
    import concourse.bacc as bacc
    import concourse.tile as tile
    from concourse import mybir

    f32 = mybir.dt.float32
    bf = mybir.dt.bfloat16
    i16 = mybir.dt.int16
    EXP = mybir.ActivationFunctionType.Exp
    MUL = mybir.AluOpType.mult
    ADD = mybir.AluOpType.add

    nc = bacc.Bacc("TRN2", target_bir_lowering=False, debug=False,
                   num_devices=NCORES)
    # The gpsimd SWDGE queue is dedicated to STORES (+ end-of-rep g2 loads):
    # stores on a load queue would FIFO-block the NEXT rep's input loads
    # behind this rep's compute, serializing reps. All input loads go on the
    # sync/scalar HWDGE queues instead.

    xT_d = nc.dram_tensor("xT", [D, S], bf, kind="ExternalInput")
    w1_d = nc.dram_tensor("w1", [D, 128], bf, kind="ExternalInput")
    wv_d = nc.dram_tensor("wv", [D, HD], bf, kind="ExternalInput")
    ab_d = nc.dram_tensor("ab", [2, 128, S], bf, kind="ExternalInput")
    mk_d = nc.dram_tensor("masks", [4, 128, SB], bf, kind="ExternalInput")
    wo_d = nc.dram_tensor("wo", [KV_DIM, D], bf, kind="ExternalInput")
    out_d = nc.dram_tensor("out", [B, SB, D], f32 if out_f32 else bf, kind="ExternalOutput")

    idn_d = nc.inline_tensor(np.eye(128, dtype=ml_dtypes.bfloat16), "idn")
    Mperm = np.zeros((128, 128), np.float32)
    for r in range(0, 32):
        Mperm[r, r + 32] = -1.0
        Mperm[r + 32, r] = 1.0
        Mperm[r + 64, r + 96] = -1.0
        Mperm[r + 96, r + 64] = 1.0
    permT_d = nc.inline_tensor(Mperm.T.astype(ml_dtypes.bfloat16), "permT")
    # selection matrices for dest-side denominator broadcast: for m-block m,
    # denmap[j, q] = rden[src(j), q] with src = 2m (rows 0:64) / 2m+1 (64:128)
    Msel = np.zeros((NCORES, 4 * 128), np.float32)
    for m in range(4):
        Msel[2 * m, 128 * m: 128 * m + HD] = 1.0
        Msel[2 * m + 1, 128 * m + HD: 128 * m + 128] = 1.0
    sel_d = nc.inline_tensor(Msel.astype(ml_dtypes.bfloat16), "sel")

    with tile.TileContext(nc) as tc:
        with (
            tc.tile_pool(name="persist", bufs=1) as pp,
            tc.tile_pool(name="work", bufs=4) as wp,
            tc.tile_pool(name="expp", bufs=4) as ep,
            tc.tile_pool(name="ps_sc", bufs=2, space="PSUM") as ps_sc,
            tc.tile_pool(name="ps_po", bufs=1, space="PSUM") as ps_po,
            tc.tile_pool(name="ps_pp", bufs=1, space="PSUM") as ps_pp,
            tc.tile_pool(name="dram", bufs=1, space="DRAM") as dp,
        ):
            # ---- persistent SBUF tensors ----
            xT = pp.tile([128, 8 * S], bf, tag="xT")          # 64KB/part
            w1 = pp.tile([128, 8 * 128], bf, tag="w1")
            wv = pp.tile([128, 8 * HD], bf, tag="wv")
            AB = pp.tile([128, 2 * S], bf, tag="AB")          # cos | sin
            mk = pp.tile([128, 4 * SB], bf, tag="mk")
            wo = pp.tile([128, 2 * D], bf, tag="wo")          # head-pair rows 128
            rot = pp.tile([128, S], bf, tag="rot")            # rows 0:64 q~, 64:128 k~
            rotk = pp.tile([HD, S], bf, tag="rotk")           # k~ at base partition 0
            rotq2 = pp.tile([128, S], bf, tag="rotq2")        # q~ dup at rows 64:128
            VS = HD + 1
            vaug = pp.tile([128, 32 * VS], bf, tag="vaug")
            outTs = []
            for q in range(NSB):
                oT = pp.tile([HD + 1, SB], bf, tag=f"outT{q}")  # row 64 = den
                outTs.append(oT)
            idn = pp.tile([128, 128], bf, tag="idn")
            permT = pp.tile([128, 128], bf, tag="permT")
            sel = pp.tile([NCORES, 4 * 128], bf, tag="sel")

            lvl = ["L", "P", "A", "C", "D"].index(stop_after)
            if lvl >= 3:
                bin_ = dp.tile([NCORES, HD + 1, 512], bf, tag="bin")
                bout = dp.tile([NCORES, HD + 1, 512], bf, tag="bout")

            At = AB[:, 0:S]
            Bt = AB[:, S:2 * S]

            # reps>1 unrolls the FULL kernel back-to-back inside one NEFF for
            # device-side timing: HW time = (T(R2)-T(R1))/(N*(R2-R1)).
            for rep in range(reps):
                # ---- input loads: batched big DMAs on 3 parallel queues ----
                xT3 = xT[:, :].rearrange("p (c s) -> p c s", c=8)
                xTd3 = xT_d[:, :].rearrange("(c p) s -> p c s", p=128)
                # sync queue: w1 first (needed by P(0)), then x slices 0,2
                w1_3 = w1[:, :].rearrange("p (c m) -> p c m", c=8)
                w1d3 = w1_d[:, :].rearrange("(c p) m -> p c m", p=128)
                nc.sync.dma_start(w1_3[:, :, :], w1d3[:, :, :])
                nc.sync.dma_start(xT3[:, :, 0:2 * SB], xTd3[:, :, 0:2 * SB])
                # scalar queue: wv/wo/masks bundle, then x slices 1,3
                wv_3 = wv[:, :].rearrange("p (c m) -> p c m", c=8)
                wvd3 = wv_d[:, :].rearrange("(c p) m -> p c m", p=128)
                nc.scalar.dma_start(wv_3[:, :, :], wvd3[:, :, :])
                wo_3 = wo[:, :].rearrange("p (c n) -> p c n", c=2)
                wod3 = wo_d[:, :].rearrange("(c p) n -> p c n", p=128)
                nc.scalar.dma_start(wo_3[:, :, :], wod3[:, :, :])
                mk3 = mk[:, :].rearrange("p (r m) -> p r m", r=4)
                nc.scalar.dma_start(mk3[:, :, :],
                                    mk_d[:, :, :].rearrange("r p m -> p r m"))
                nc.scalar.dma_start(xT3[:, :, 2 * SB:4 * SB],
                                    xTd3[:, :, 2 * SB:4 * SB])
                # cos on sync, sin on scalar (gpsimd carries no loads)
                nc.sync.dma_start(AB[:, 0:S], ab_d[0, :, :])
                nc.scalar.dma_start(AB[:, S:2 * S], ab_d[1, :, :])
                nc.sync.dma_start(idn[:, :], idn_d[:, :])
                nc.sync.dma_start(permT[:, :], permT_d[:, :])
                nc.sync.dma_start(sel[:, :], sel_d[:, :])
                nc.sync.dma_start(xT3[:, :, 4 * SB:6 * SB], xTd3[:, :, 4 * SB:6 * SB])
                nc.scalar.dma_start(xT3[:, :, 6 * SB:8 * SB], xTd3[:, :, 6 * SB:8 * SB])
                # one memset covers all 32 ones-columns; V copies overwrite the rest
                nc.vector.memset(vaug[:, :], 1.0)

                if lvl < 1:
                    nc.gpsimd.dma_start(out_d[0, 0:128, 0:1024], xT[:, 0:1024])  # gpsimd casts if needed

                pat = EXP_PATTERN if exp_pattern is None else exp_pattern
                exp_gidx = 0
                # ---- stages P+A interleaved ----
                # P(j) is emitted as PIECES (qk-proj / perm / V-halves /
                # transposes) interleaved between A(j-1)'s groups, so the
                # in-order PE queue has independent matmul work wherever a
                # cross-engine dependency (DVE copy) would otherwise stall it.
                # P(j) for even j does V projection for blocks j and j+1
                # col-tiled: rows 0:64 = V(j), rows 64:128 = V(j+1).

                def xs(d8, _j):
                    return xT[:, S * d8 + SB * _j: S * d8 + SB * (_j + 1)]

                stash = {}

                def make_P_pieces(j):
                    def piece_qk(_j=j):
                        sc = ps_pp.tile([128, SB], f32, tag="pp")
                        for d8 in range(8):
                            nc.tensor.matmul(sc[:, :], w1[:, 128 * d8: 128 * (d8 + 1)],
                                             xs(d8, _j), start=(d8 == 0), stop=(d8 == 7))
                        p1s = wp.tile([128, SB], bf, tag="p1s")
                        nc.vector.tensor_copy(p1s[:, :], sc[:, :])
                        stash[("p1s", _j)] = p1s

                    def piece_perm(_j=j):
                        p1s = stash.pop(("p1s", _j))
                        sc2 = ps_pp.tile([128, SB], f32, tag="pp")
                        nc.tensor.matmul(sc2[:, :], permT[:, :], p1s[:, :],
                                         start=True, stop=True)
                        u = wp.tile([128, SB], bf, tag="u")
                        w_ = wp.tile([128, SB], f32, tag="w_")
                        nc.vector.tensor_mul(u[:, :], p1s[:, :], At[:, SB * _j: SB * (_j + 1)])
                        nc.vector.tensor_mul(w_[:, :], sc2[:, :], Bt[:, SB * _j: SB * (_j + 1)])
                        nc.vector.tensor_add(rot[:, SB * _j: SB * (_j + 1)], u[:, :], w_[:, :])
                        # k~ copy down to base partition 0; q~ dup up to 64:128
                        nc.gpsimd.dma_start(rotk[:, SB * _j: SB * (_j + 1)],
                                            rot[64:128, SB * _j: SB * (_j + 1)])
                        nc.gpsimd.dma_start(rotq2[64:128, SB * _j: SB * (_j + 1)],
                                            rot[0:64, SB * _j: SB * (_j + 1)])

                    def piece_v(half, _j=j):
                        if half == 0:
                            pv = ps_pp.tile([128, SB], f32, tag="pp")
                            stash[("pv", _j)] = pv
                            for d8 in range(8):
                                nc.tensor.matmul(pv[0:HD, :], wv[:, HD * d8: HD * (d8 + 1)],
                                                 xs(d8, _j), start=(d8 == 0), stop=(d8 == 7),
                                                 tile_position=(0, 0))
                        else:
                            pv = stash[("pv", _j)]
                            for d8 in range(8):
                                nc.tensor.matmul(pv[HD:128, :], wv[:, HD * d8: HD * (d8 + 1)],
                                                 xs(d8, _j + 1), start=(d8 == 0), stop=(d8 == 7),
                                                 tile_position=(0, 64))
                            vts = wp.tile([128, SB], bf, tag="vts")
                            nc.vector.tensor_copy(vts[:, :], pv[:, :])
                            stash.pop(("pv", _j))
                            stash[("vts", _j)] = vts

                    def piece_tr(t, _j=j):
                        vts = stash[("vts", _j)]
                        tr = ps_pp.tile([128, 128], bf, tag="pp")
                        nc.tensor.transpose(tr[:, :], vts[:, 128 * t: 128 * (t + 1)],
                                            idn[:, :])
                        ca = 4 * _j + t
                        cb = 4 * (_j + 1) + t
                        nc.vector.tensor_copy(vaug[:, VS * ca: VS * ca + HD],
                                              tr[:, 0:HD])
                        nc.vector.tensor_copy(vaug[:, VS * cb: VS * cb + HD],
                                              tr[:, HD:128])
                        if t == 3:
                            stash.pop(("vts", _j))

                    pieces = [piece_qk, piece_perm]
                    if j % 2 == 0:
                        pieces += [lambda _=None: piece_v(0), lambda _=None: piece_v(1)]
                        pieces += [(lambda _t: (lambda _=None: piece_tr(_t)))(t)
                                   for t in range(4)]
                    return pieces

                if lvl == 1:
                    for j in range(NSB):
                        for pc in make_P_pieces(j):
                            pc()

                # P(0) fully upfront (A(0) needs rot(0) and V chunks 0..3)
                if lvl >= 2:
                    for pc in make_P_pieces(0):
                        pc()

                for j in range(NSB if lvl >= 2 else 0):
                    pend = make_P_pieces(j + 1) if j + 1 < NSB else []
                    # ---- A(qb=j) ----
                    # Software-pipelined: PE emission order is
                    # sc(0), sc(1), AV(0), sc(2), AV(1), ... so the in-order
                    # PE queue never stalls on exp(g) while scores(g+1) could
                    # run; exp(g) overlaps scores(g+1) on PE.
                    qb = j
                    po = ps_po.tile([HD + 1, 512], f32, tag="po")
                    nk = 4 * (qb + 1)
                    groups = [(g0, min(GRP, nk - g0)) for g0 in range(0, nk, GRP)]

                    def emit_scores(g0, cnt, _qb=qb):
                        sc = ps_sc.tile([128, GRP * SB], f32, tag="sc")
                        for r in range(cnt):
                            kb = g0 + r
                            dst = sc[:, 512 * r: 512 * (r + 1)]
                            if r % 2 == 0:
                                # row-tile T0: k~/q~ from partitions 0:64
                                nc.tensor.matmul(dst, rotk[:, 128 * kb: 128 * (kb + 1)],
                                                 rot[0:HD, SB * _qb: SB * (_qb + 1)],
                                                 start=True, stop=True)
                            else:
                                # row-tile T8: concurrent with the T0 matmul
                                nc.tensor.matmul(dst, rot[64:128, 128 * kb: 128 * (kb + 1)],
                                                 rotq2[64:128, SB * _qb: SB * (_qb + 1)],
                                                 start=True, stop=True)
                        return sc

                    def emit_exp(sc, g0, cnt):
                        nonlocal exp_gidx
                        pe = ep.tile([128, GRP * SB], bf, tag="pe")
                        use_act = pat[exp_gidx % len(pat)]
                        exp_gidx += 1
                        if use_act:
                            nc.scalar.activation(pe[:, 0:512 * cnt], sc[:, 0:512 * cnt],
                                                 EXP)
                        else:
                            # bf16 Schraudolph on DVE: int16 round bitcast bf16
                            nc.vector.tensor_scalar(
                                pe[:, 0:512 * cnt].bitcast(i16), sc[:, 0:512 * cnt],
                                SCH_C1, SCH_C2, MUL, ADD)
                        for r in range(cnt):
                            di = (g0 + r) - (nk - 4)
                            if di >= 0:
                                mw = 128 * (di + 1)  # all-ones beyond this col
                                nc.vector.tensor_mul(pe[:, 512 * r: 512 * r + mw],
                                                     pe[:, 512 * r: 512 * r + mw],
                                                     mk[:, SB * di: SB * di + mw])
                        return pe

                    def emit_av(pe, g0, cnt):
                        for r in range(cnt):
                            kb = g0 + r
                            nc.tensor.matmul(po[:, :],
                                             vaug[:, VS * kb: VS * kb + HD + 1],
                                             pe[:, 512 * r: 512 * (r + 1)],
                                             start=(kb == 0), stop=(kb == nk - 1))

                    if pipelined:
                        sc_cur = emit_scores(*groups[0])
                        for gi, (g0, cnt) in enumerate(groups):
                            pe_cur = emit_exp(sc_cur, g0, cnt)
                            if gi + 1 < len(groups):
                                sc_cur = emit_scores(*groups[gi + 1])
                            if pend:
                                # one P(j+1) piece per group boundary: its PE
                                # work runs while exp(gi) is still in flight
                                pend.pop(0)()
                            emit_av(pe_cur, g0, cnt)
                    else:
                        for g0, cnt in groups:
                            sc_cur = emit_scores(g0, cnt)
                            pe_cur = emit_exp(sc_cur, g0, cnt)
                            emit_av(pe_cur, g0, cnt)
                    while pend:
                        pend.pop(0)()
                    # UNNORMALIZED bounce: numerator rows 0:64 + den row 64
                    # travel through the a2a; normalization happens dest-side
                    # in stage D (keeps A-phase DVE free for exp work).
                    # psum->sbuf copy alternates ACT/DVE to balance engines.
                    nc.vector.tensor_copy(outTs[qb][:, :], po[:, :])
                    if lvl >= 3:
                        # eager bounce-out: slice qb of outT is exactly a2a block qb
                        nc.gpsimd.dma_start(bin_[qb, :, :], outTs[qb][:, :])

                if lvl == 1:
                    nc.gpsimd.dma_start(out_d[0, 0:128, 0:1024],
                                        rot[:, 0:1024].bitcast(bf))
                    nc.gpsimd.dma_start(out_d[1, 0:128, 0:1024],
                                        vaug[:, 0:1024].bitcast(bf))
                if lvl == 2:
                    nc.gpsimd.dma_start(out_d[0, 0:65, 0:512], outTs[0][:, :])
                    nc.gpsimd.dma_start(out_d[0, 0:65, 512:1024], outTs[1][:, :])

                # ---- stage C: AllToAll (512-col output slices across 8 cores) ----
                if lvl >= 3:
                    if collective:
                        from concourse import mybir as _mb
                        nc.gpsimd.collective_compute(
                            "AllToAll", _mb.AluOpType.bypass,
                            replica_groups=[list(range(NCORES))],
                            ins=[bin_.opt()], outs=[bout.opt()],
                        )
                    else:
                        # single-core timeline-sim stand-in: local DRAM->DRAM move
                        nc.sync.dma_start(bout[:, :, :], bin_[:, :, :])
                    if lvl == 3:
                        nc.gpsimd.dma_start(out_d[0, 0:64, 0:512], bout[0, :, :])

                # ---- stage D: wo matmul + output ----
                if lvl >= 4:
                    # head pairs stacked on partition halves -> K=128 wo matmuls
                    g2 = pp.tile([128, 4 * 512], bf, tag="g2")
                    g2lo = g2[0:HD, :].rearrange("p (m s) -> p m s", m=4)
                    g2hi = g2[HD:128, :].rearrange("p (m s) -> p m s", m=4)
                    bt3 = bout[:, 0:HD, :].rearrange("(m e) p s -> e p m s", e=2)
                    nc.gpsimd.dma_start(g2lo[:, :, :], bt3[0, :, :, :])
                    nc.gpsimd.dma_start(g2hi[:, :, :], bt3[1, :, :, :])
                    # dest-side normalization: gather den rows [src, q], recip,
                    # matmul-broadcast via selection matrices, divide g2
                    g2den = pp.tile([NCORES, 512], bf, tag="g2den")
                    nc.gpsimd.dma_start(g2den[:, :], bout[:, HD, :])
                    rden = wp.tile([NCORES, 512], bf, tag="rden")
                    with nc.allow_low_precision(reason="bf16 recip-den; 0.4% rel, within gate"):
                        nc.vector.reciprocal(rden[:, :], g2den[:, :])
                    for m in range(4):
                        dm = ps_pp.tile([128, 512], f32, tag="pp")
                        nc.tensor.matmul(dm[:, :], sel[:, 128 * m: 128 * (m + 1)],
                                         rden[:, :], start=True, stop=True)
                        nc.vector.tensor_mul(g2[:, 512 * m: 512 * (m + 1)],
                                             g2[:, 512 * m: 512 * (m + 1)], dm[:, :])
                    for b in range(B):
                        for t in range(4):
                            ys = wp.tile([128, 1024], bf, tag="ys")
                            for nh in range(2):
                                yp = ps_sc.tile([128, 512], f32, tag="sc")
                                for pr in range(2):
                                    m = 2 * b + pr
                                    nc.tensor.matmul(yp[:, :],
                                                     g2[:, 512 * m + 128 * t: 512 * m + 128 * (t + 1)],
                                                     wo[:, D * pr + 512 * nh: D * pr + 512 * (nh + 1)],
                                                     start=(pr == 0), stop=(pr == 1))
                                # stage psum->sbuf alternating DVE / ACT (both idle here)
                                if (2 * (4 * b + t) + nh) % 2 == 0:
                                    nc.vector.tensor_copy(ys[:, 512 * nh: 512 * (nh + 1)], yp[:, :])
                                else:
                                    nc.scalar.copy(ys[:, 512 * nh: 512 * (nh + 1)], yp[:, :])
                            nc.gpsimd.dma_start(out_d[b, 128 * t: 128 * (t + 1), :],
                                                ys[:, :])

    nc.compile()
    return nc


def _get_nc(reps=1):
    key = ("nc", reps)
    if key not in _CACHE:
        _CACHE[key] = _build_nc(reps=reps)
    return _CACHE[key]


def _prep_in_maps(x, wq, wk, wv, wo, freq_cos, freq_sin):
    x = np.asarray(x, np.float32)
    wq = np.asarray(wq, np.float32)
    wk = np.asarray(wk, np.float32)
    wv = np.asarray(wv, np.float32)
    wo = np.asarray(wo, np.float32)
    cos = np.asarray(freq_cos, np.float32)
    sin = np.asarray(freq_sin, np.float32)

    # group-sum wq per kv head (einsum sums over group axis); fold both /8 scales
    wqr = wq.reshape(D, QH, HD)
    wq_eff = np.stack([wqr[:, h::KVH].sum(axis=1) for h in range(KVH)], axis=1) / 64.0
    wkr = wk.reshape(D, KVH, HD)
    W1 = np.empty((KVH, D, 128), np.float32)
    for h in range(KVH):
        q1, q2 = wq_eff[:, h, 0::2], wq_eff[:, h, 1::2]
        k1, k2 = wkr[:, h, 0::2], wkr[:, h, 1::2]
        W1[h] = np.concatenate([q1, q2, k1, k2], axis=1)
    Wv = np.ascontiguousarray(wv.reshape(D, KVH, HD).transpose(1, 0, 2))

    A = np.tile(cos.T, (4, 1)).astype(ml_dtypes.bfloat16)   # [128, S]
    Bm = np.tile(sin.T, (4, 1)).astype(ml_dtypes.bfloat16)
    ab = np.ascontiguousarray(np.stack([A, Bm]))

    qi = np.arange(SB)[None, :]
    ki = np.arange(128)[:, None]
    masks = np.ascontiguousarray(
        np.stack([(qi >= ki + 128 * r) for r in range(4)]).astype(ml_dtypes.bfloat16))

    xTb = [np.ascontiguousarray(x[b].T).astype(ml_dtypes.bfloat16) for b in range(B)]

    in_maps = []
    for c in range(NCORES):
        b, h = c // KVH, c % KVH
        in_maps.append({
            "xT": xTb[b],
            "w1": np.ascontiguousarray(W1[h]).astype(ml_dtypes.bfloat16),
            "wv": np.ascontiguousarray(Wv[h]).astype(ml_dtypes.bfloat16),
            "ab": ab,
            "masks": masks,
            "wo": wo.astype(ml_dtypes.bfloat16),
        })
    return in_maps


def _assemble(results):
    full = np.empty((B, S, D), np.float32)
    for c in range(NCORES):
        y = results[c]["out"]  # [B, 512, D] bf16 or f32
        for b in range(B):
            full[b, SB * c: SB * (c + 1), :] = y[b].astype(np.float32)
    return full


def _ensure_axon_hooks_stub():
    # slim axon builds lack antenv.axon_hooks; degrade trace=True gracefully
    try:
        import antenv.axon_hooks  # noqa: F401
    except Exception:
        import types
        m = types.ModuleType("antenv.axon_hooks")
        m.get_axon_ntff_profile_hook = lambda: None
        sys.modules["antenv.axon_hooks"] = m


def run(in_maps, trace=False):
    from concourse.bass_utils import run_bass_kernel_spmd
    _ensure_axon_hooks_stub()
    nc = _get_nc()
    res = run_bass_kernel_spmd(nc, in_maps, core_ids=list(range(NCORES)),
                               trace=trace)
    return res


def kernel(**inputs):
    in_maps = _prep_in_maps(**inputs)
    res = run(in_maps, trace=False)
    return _assemble(res.results)


if __name__ == "__main__":
    # smoke: build only
    _get_nc()
    print("built ok")
